# revision 49
# baseline (speedup 1.0000x reference)
# Trainium2 Bass kernel for the CLOSEgaps-style GNN message-passing module.
#
# Math (per head h, x0 = node_features):
#   deg   = inc.sum(1) + EPS_AGG                          [n]
#   tn    = x @ Wn[h] + bn[h]                             [n, H]
#   te    = ef @ We[h] + be[h]                            [E, H]
#   agg   = (inc @ te) / deg                              [n, H]
#   score = lrelu((tn + agg) @ Wa[h] + ba[h], 0.2)        [n, 1]
#   coeff = sigmoid(score)
#   upd   = coeff * agg + tn
#   out   = minmax(upd @ Wo[h] + bo[h]);  x = relu(out)
#
# Key reassociations (exact in real arithmetic):
#   inc @ te = P @ We[h] + rowsum(inc) x be[h],   P := inc @ ef  (computed ONCE)
#   out = coeff*(agg @ Wo) + tn @ Wo + bo
#   agg @ Wo = rdeg*(P @ (We@Wo)) + (1-eps*rdeg) x (be@Wo)
#   tn  @ Wo = x @ (Wn@Wo) + (bn@Wo)
#   (tn+agg) @ Wa = x @ (Wn@Wa) + rdeg*(P @ (We@Wa)) + consts
# The [128,128] / [128,1] fused weights (Wn@Wo etc.) are precomputed on host.
# All biases are zero in this problem's setup_inputs(); if any bias is nonzero
# we fall back to an exact numpy implementation.
#
# Sharding: nodes row-sharded 8 ways (1024 rows of inc / node_features per
# core); edge_features + weights replicated. Per-head min/max is a [128,2]
# AllGather across the 8 cores.
#
# On-device layout is feature-major ("transposed"): xT[d, m], PT[d, m],
# outT[o, m] with m (node) on the free axis, so the per-feature min/max is a
# free-axis reduce and the per-node coeff broadcast is a K=1 matmul.

import os
import numpy as np

N_CORES = 8
N_NODES, N_EDGES = 8192, 4096
D, H, O, NH = 128, 256, 128, 4
M = N_NODES // N_CORES          # 1024 nodes per core
MT = M // 128                   # 8 node tiles per core
MG = 2                          # 2 m-groups of 512
EC = N_EDGES // 128             # 32 edge chunks
EPS_AGG = 1e-8
EPS_MM = 1e-8
NEG_HUGE = -3.0e38

_CACHE = {}


def _build_bass():
    import concourse.bass as bass
    import concourse.mybir as mybir
    import concourse.tile as tile
    from concourse import bacc
    from concourse.masks import make_identity

    f32 = mybir.dt.float32
    f32r = mybir.dt.float32r
    AF = mybir.ActivationFunctionType
    ALU = mybir.AluOpType

    # Bacc (not plain Bass): its compile pipeline splits multi-wait sync
    # into EventSemaphore instructions (HW allows 1 wait per instruction)
    nc = bacc.Bacc("TRN2", target_bir_lowering=False, num_devices=N_CORES)

    inc_d = nc.dram_tensor("inc", [M, N_EDGES], f32r, kind="ExternalInput")
    nf_d = nc.dram_tensor("nf", [M, D], f32, kind="ExternalInput")
    ef_d = nc.dram_tensor("ef", [N_EDGES, D], f32r, kind="ExternalInput")
    wno_d = nc.dram_tensor("wno", [NH, D, O], f32r, kind="ExternalInput")
    weo_d = nc.dram_tensor("weo", [NH, D, O], f32r, kind="ExternalInput")
    wna_d = nc.dram_tensor("wna", [NH, D], f32r, kind="ExternalInput")
    wea_d = nc.dram_tensor("wea", [NH, D], f32r, kind="ExternalInput")
    out_d = nc.dram_tensor("out", [M, D], f32, kind="ExternalOutput")
    RG = [list(range(N_CORES))]

    with tile.TileContext(nc) as tc:
        # ---- persistent pools -------------------------------------------
        consts = tc.alloc_tile_pool(name="consts", bufs=1)
        wpool = tc.alloc_tile_pool(name="wpool", bufs=1)
        xpool = tc.alloc_tile_pool(name="xpool", bufs=2)
        persist = tc.alloc_tile_pool(name="persist", bufs=1)
        headsb = tc.alloc_tile_pool(name="headsb", bufs=1)
        dram = tc.alloc_tile_pool(name="dram", bufs=2, space="DRAM")

        ident = consts.tile([128, 128], f32, name="ident")
        make_identity(nc, ident)
        # memset can't write f32r; write f32 ones and cast-copy on ACT
        ones_f32a = consts.tile([1, 128], f32, name="ones_f32a")
        nc.vector.memset(ones_f32a, 1.0)
        ones_col = consts.tile([1, 128], f32r, name="ones_col")
        nc.scalar.copy(ones_col, ones_f32a)
        # f32r identity for the incidence transposes (1.5 vs 2.0 cyc/row);
        # produced by an ACT copy so the f32r producer-chain rule is satisfied
        ident_r = consts.tile([128, 128], f32r, name="ident_r")
        nc.scalar.copy(ident_r, ident)

        # fused weights, feature-major
        wno_sb = wpool.tile([128, NH, O], f32r, name="wno_sb")
        nc.sync.dma_start(out=wno_sb, in_=wno_d[:, :, :].rearrange("h d o -> d h o"))
        weo_sb = wpool.tile([128, NH, O], f32r, name="weo_sb")
        nc.sync.dma_start(out=weo_sb, in_=weo_d[:, :, :].rearrange("h d o -> d h o"))
        wna_sb = wpool.tile([128, NH], f32r, name="wna_sb")
        nc.sync.dma_start(out=wna_sb, in_=wna_d[:, :].rearrange("h d -> d h"))
        wea_sb = wpool.tile([128, NH], f32r, name="wea_sb")
        nc.sync.dma_start(out=wea_sb, in_=wea_d[:, :].rearrange("h d -> d h"))

        nf_nat = wpool.tile([128, MT, D], f32, name="nf_nat")
        nc.sync.dma_start(out=nf_nat, in_=nf_d[:, :].rearrange("(t p) d -> p t d", p=128))

        ef_sb = wpool.tile([128, EC, D], f32r, name="ef_sb")
        nc.sync.dma_start(out=ef_sb, in_=ef_d[:, :].rearrange("(c p) d -> p c d", p=128))

        PTn = persist.tile([128, M], f32r, name="PTn")           # (P/deg)^T
        PTu = persist.tile([128, M], f32, name="PTu")            # P^T unnorm
        rdeg_row = persist.tile([1, M], f32r, name="rdeg_row")   # 1/deg as row
        rb_sb = persist.tile([128, M], f32, name="rb_sb")       # rdeg bcast to 128 p

        xT = xpool.tile([128, M], f32r, name="xT", tag="xT")

        # ---- setup phase: load, transpose, P = inc @ ef ------------------
        with tc.tile_pool(name="setup_sb", bufs=1) as ssb, \
             tc.tile_pool(name="nat_p", bufs=3) as natp, \
             tc.tile_pool(name="psTP", bufs=2, space="PSUM") as psTP, \
             tc.tile_pool(name="psPT", bufs=2, space="PSUM") as psPT, \
             tc.tile_pool(name="psRB", bufs=1, space="PSUM") as psRB:

            # x0^T via PE transposes, copied out 8 tiles at a time
            tp = psTP.tile([128, 1024], f32, name="tp", tag="tp")
            for k in range(8):
                nc.tensor.transpose(tp[:, k * 128:(k + 1) * 128],
                                    nf_nat[:, k, :], ident)
            nc.scalar.copy(xT, tp)

            # incidence: per m-group of 4 tiles -> transpose -> P matmul
            incT = ssb.tile([128, EC, 512], f32r, name="incT")   # one m-group
            degC = ssb.tile([128, MT], f32, name="degC")
            rdegC = ssb.tile([128, MT], f32, name="rdegC")
            for g in range(MG):
                gs = slice(g * 512, (g + 1) * 512)
                for tl in range(4):
                    t = g * 4 + tl
                    nat = natp.tile([128, N_EDGES], f32r, name="nat", tag="nat")
                    nc.sync.dma_start(out=nat,
                                      in_=inc_d[t * 128:(t + 1) * 128, :])
                    # deg rowsum: fold twice at DVE 2x (all-SBUF
                    # tensor_tensor), then a 1024-wide 1x reduce - ~40%
                    # cheaper than one 4096-wide 1x reduce
                    f1 = ssb.tile([128, 2048], f32, name="fold1",
                                  tag="fold1", bufs=2)
                    nc.vector.tensor_add(f1, nat[:, 0:2048].bitcast(f32),
                                         nat[:, 2048:4096].bitcast(f32))
                    nc.vector.tensor_add(f1[:, 0:1024], f1[:, 0:1024],
                                         f1[:, 1024:2048])
                    nc.vector.tensor_reduce(
                        degC[:, t:t + 1], f1[:, 0:1024],
                        axis=mybir.AxisListType.X, op=ALU.add)
                    for c8 in range(EC // 8):
                        tp = psTP.tile([128, 1024], f32r, name="tp", tag="tp")
                        for k in range(8):
                            c = c8 * 8 + k
                            nc.tensor.transpose(
                                tp[:, k * 128:(k + 1) * 128],
                                nat[:, c * 128:(c + 1) * 128], ident_r)
                        dst = incT[:, c8 * 8:(c8 + 1) * 8,
                                   tl * 128:(tl + 1) * 128]
                        srcv = tp.rearrange("p (k m) -> p k m", k=8)
                        # 2/3 of copies on ACT: DVE also carries the deg
                        # folds and PTn multiplies in this phase
                        if c8 % 3 != 2:
                            nc.scalar.copy(dst, srcv)
                        else:
                            nc.vector.tensor_copy(dst, srcv)

                    # P^T accumulation for the finished 256-col pair: PE
                    # alternates transposes and matmuls so the ACT/DVE copies
                    # drain behind the P matmuls instead of stalling PE
                    if tl % 2 == 1:
                        pr = (tl - 1) // 2
                        ms = slice(pr * 256, (pr + 1) * 256)
                        ptp = psPT.tile([128, 256], f32, name="ptp", tag="pt")
                        for c in range(EC):
                            nc.tensor.matmul(
                                ptp, ef_sb[:, c, :], incT[:, c, ms],
                                start=(c == 0), stop=(c == EC - 1))
                        prs = slice(g * 512 + pr * 256, g * 512 + (pr + 1) * 256)
                        nc.scalar.copy(PTu[:, prs], ptp)

            # one rdeg pass for all 8 node tiles (reciprocal has a large
            # per-call cost; 8 per-pair calls measured ~20us of DVE time)
            nc.vector.tensor_scalar_add(rdegC, degC, EPS_AGG)
            nc.vector.reciprocal(rdegC, rdegC)
            for g2 in range(MG):
                g2s = slice(g2 * 512, (g2 + 1) * 512)
                tpr = psRB.tile([1, 512], f32, name="tpr", tag="tpr")
                for k in range(4):
                    j = g2 * 4 + k
                    nc.tensor.transpose(tpr[0:1, k * 128:(k + 1) * 128],
                                        rdegC[:, j:j + 1], ident)
                with nc.allow_low_precision(reason="rdeg feeds fp32r matmul"):
                    nc.scalar.copy(rdeg_row[0:1, g2s], tpr)
                rbp = psRB.tile([128, 512], f32, name="rbp", tag="rb")
                nc.tensor.matmul(rbp, ones_col, rdeg_row[0:1, g2s],
                                 start=True, stop=True)
                nc.scalar.copy(rb_sb[:, g2s], rbp)
                # all-SBUF multiply runs in the DVE 2x mode
                nc.vector.tensor_tensor(
                    out=PTn[:, g2s], in0=PTu[:, g2s], in1=rb_sb[:, g2s],
                    op=ALU.mult)

        # ---- head phase --------------------------------------------------
        Gn = headsb.tile([128, M], f32, name="Gn")
        lr_row = headsb.tile([1, M], f32, name="lr_row")
        outs = headsb.tile([128, M], f32, name="outs")
        h1a = headsb.tile([128, 512], f32, name="h1a", bufs=2, tag="h1")
        coeff_row = headsb.tile([1, M], f32r, name="coeff_row")
        mm_sb = headsb.tile([128, 2], f32, name="mm_sb", bufs=2, tag="mm_sb")
        mm_all = headsb.tile([128, N_CORES, 2], f32, name="mm_all", bufs=2,
                             tag="mm_all")
        gmn = headsb.tile([128, 1], f32, name="gmn", bufs=2, tag="gmn")
        srg = headsb.tile([128, 1], f32, name="srg", bufs=2, tag="srg")
        sct = headsb.tile([128, 1], f32, name="sct", bufs=2, tag="sct")
        nbt = headsb.tile([128, 1], f32, name="nbt", bufs=2, tag="nbt")

        with tc.tile_pool(name="psT2", bufs=2, space="PSUM") as psT2, \
             tc.tile_pool(name="psSC", bufs=2, space="PSUM") as psSC, \
             tc.tile_pool(name="psCB", bufs=2, space="PSUM") as psCB, \
             tc.tile_pool(name="psGN", bufs=2, space="PSUM") as psGN:

            n_heads = int(os.environ.get("BGNN_HEADS", str(NH)))
            for h in range(n_heads):
                # Gn = Weo[h]^T @ PTn  (x-independent; overlaps prior AG)
                for g in range(MG):
                    gs = slice(g * 512, (g + 1) * 512)
                    gnp = psGN.tile([128, 512], f32, name="gnp", tag="gn")
                    nc.tensor.matmul(gnp, weo_sb[:, h, :], PTn[:, gs],
                                     start=True, stop=True)
                    nc.scalar.copy(Gn[:, gs], gnp)

                t2ps = []
                for g in range(MG):
                    gs = slice(g * 512, (g + 1) * 512)
                    # T2 = Wno[h]^T @ xT
                    t2p = psT2.tile([128, 512], f32, name="t2p", tag="t2")
                    nc.tensor.matmul(t2p, wno_sb[:, h, :], xT[:, gs],
                                     start=True, stop=True)
                    t2ps.append(t2p)
                    # score = Wna[h]^T @ xT + Wea[h]^T @ PTn
                    scp = psSC.tile([1, 512], f32, name="scp", tag="sc")
                    nc.tensor.matmul(scp, wna_sb[:, h:h + 1], xT[:, gs],
                                     start=True, stop=False)
                    nc.tensor.matmul(scp, wea_sb[:, h:h + 1], PTn[:, gs],
                                     start=False, stop=True)
                    # coeff = sigmoid(lrelu(score, 0.2)); lrelu == max(x, 0.2x)
                    nc.vector.tensor_scalar_mul(lr_row[0:1, gs], scp, 0.2)
                    nc.vector.tensor_tensor(
                        out=lr_row[0:1, gs], in0=scp, in1=lr_row[0:1, gs],
                        op=ALU.max)
                    nc.scalar.activation(coeff_row[0:1, gs], lr_row[0:1, gs],
                                         AF.Sigmoid)

                for g in range(MG):
                    gs = slice(g * 512, (g + 1) * 512)
                    # broadcast coeff across partitions, K=1 matmul
                    cbp = psCB.tile([128, 512], f32, name="cbp", tag="cb")
                    nc.tensor.matmul(cbp, ones_col, coeff_row[0:1, gs],
                                     start=True, stop=True)
                    # outT = coeff_b * Gn + T2
                    # (tensor_tensor_reduce would fuse the max, but it
                    # hard-crashes the device - NRT_EXEC_UNIT_UNRECOVERABLE)
                    nc.vector.tensor_tensor(
                        out=h1a, in0=cbp, in1=Gn[:, gs], op=ALU.mult)
                    nc.vector.tensor_tensor(
                        out=outs[:, gs], in0=t2ps[g], in1=h1a, op=ALU.add)

                # local min / max -> [128, 2]
                nc.vector.tensor_reduce(
                    mm_sb[:, 0:1], outs, axis=mybir.AxisListType.X, op=ALU.min)
                nc.vector.tensor_reduce(
                    mm_sb[:, 1:2], outs, axis=mybir.AxisListType.X, op=ALU.max)

                no_cc = bool(int(os.environ.get("BGNN_NO_CC", "0")))
                if not no_cc:
                    # cross-core AllGather of [128, 2]
                    mm_in = dram.tile([128, 2], f32, name="mm_in", tag="mm_in")
                    nc.sync.dma_start(out=mm_in, in_=mm_sb)
                    mm_out = dram.tile([N_CORES * 128, 2], f32, name="mm_out",
                                       tag="mm_out")
                    nc.gpsimd.collective_compute(
                        "AllGather", ALU.bypass,
                        replica_groups=RG,
                        ins=[mm_in.opt()],
                        outs=[mm_out.opt()])
                    nc.sync.dma_start(
                        out=mm_all,
                        in_=mm_out[:, :].rearrange("(r p) c -> p r c", p=128))

                    # global min/max -> scale + bias for normalize+relu
                    nc.vector.tensor_reduce(
                        gmn, mm_all[:, :, 0], axis=mybir.AxisListType.X,
                        op=ALU.min)
                    nc.vector.tensor_reduce(
                        srg, mm_all[:, :, 1], axis=mybir.AxisListType.X,
                        op=ALU.max)
                else:
                    nc.vector.tensor_copy(gmn, mm_sb[:, 0:1])
                    nc.vector.tensor_copy(srg, mm_sb[:, 1:2])
                nc.vector.tensor_sub(srg, srg, gmn)
                nc.vector.tensor_scalar_add(srg, srg, EPS_MM)
                nc.vector.reciprocal(sct, srg)
                # nb = -gmn * s
                nc.vector.scalar_tensor_tensor(
                    out=nbt, in0=gmn, scalar=-1.0, in1=sct,
                    op0=ALU.mult, op1=ALU.mult)

                # x_next = relu(outT * s + nb), per-partition scale/bias
                # (last head's x is only transposed back, never matmul input)
                xdt = f32r if h < NH - 1 else f32
                xT_next = xpool.tile([128, M], xdt, name="xT_next", tag="xT")
                nc.scalar.activation(xT_next, outs, AF.Relu,
                                     bias=nbt, scale=sct)
                xT = xT_next

        # ---- final: transpose back to node-major and store --------------
        with tc.tile_pool(name="psF", bufs=2, space="PSUM") as psF, \
             tc.tile_pool(name="fout", bufs=2) as fout:
            for t4 in range(2):
                fp = psF.tile([128, 512], f32, name="fp", tag="fp")
                for k in range(4):
                    t = t4 * 4 + k
                    srcap = xT[:, t * 128:(t + 1) * 128]
                    if srcap.dtype != f32:
                        srcap = srcap.bitcast(f32)
                    nc.tensor.transpose(fp[:, k * 128:(k + 1) * 128], srcap,
                                        ident)
                onat = fout.tile([128, 512], f32, name="onat", tag="onat")
                nc.scalar.copy(onat, fp)
                nc.sync.dma_start(
                    out=out_d[t4 * 512:(t4 + 1) * 512, :]
                        .rearrange("(k p) d -> p k d", p=128),
                    in_=onat.rearrange("p (k d) -> p k d", k=4))

        dram.release()
        headsb.release()
        persist.release()
        xpool.release()
        wpool.release()
        consts.release()

    nc.finalize()
    return nc


def _numpy_fallback(node_features, incidence_matrix, edge_features,
                    Wn, bn, We, be, Wa, ba, Wo, bo):
    def lrelu(x):
        return np.where(x >= 0, x, 0.2 * x)

    def sigmoid(x):
        return 1.0 / (1.0 + np.exp(-x))

    inc = incidence_matrix.astype(np.float32)
    deg = inc.sum(axis=1, keepdims=True) + EPS_AGG
    x = node_features.astype(np.float32)
    for h in range(NH):
        tn = x @ Wn[h] + bn[h]
        te = edge_features @ We[h] + be[h]
        agg = (inc @ te) / deg
        score = lrelu((tn + agg) @ Wa[h] + ba[h])
        coeff = sigmoid(score)
        upd = coeff * agg + tn
        out = upd @ Wo[h] + bo[h]
        mn = out.min(axis=0, keepdims=True)
        mx = out.max(axis=0, keepdims=True)
        out = (out - mn) / (mx - mn + EPS_MM)
        x = np.maximum(out, 0.0)
    return x.astype(np.float32)


def kernel(node_features, incidence_matrix, edge_features,
           Wn, bn, We, be, Wa, ba, Wo, bo):
    node_features = np.asarray(node_features, dtype=np.float32)
    incidence_matrix = np.asarray(incidence_matrix, dtype=np.float32)
    edge_features = np.asarray(edge_features, dtype=np.float32)
    Wn, bn = np.asarray(Wn, np.float32), np.asarray(bn, np.float32)
    We, be = np.asarray(We, np.float32), np.asarray(be, np.float32)
    Wa, ba = np.asarray(Wa, np.float32), np.asarray(ba, np.float32)
    Wo, bo = np.asarray(Wo, np.float32), np.asarray(bo, np.float32)

    if any(np.any(b) for b in (bn, be, ba, bo)):
        # device fast-path folds the (identically zero) bias terms away
        return _numpy_fallback(node_features, incidence_matrix, edge_features,
                               Wn, bn, We, be, Wa, ba, Wo, bo)

    from concourse.bass_utils import run_bass_kernel_spmd

    if "nc" not in _CACHE:
        _CACHE["nc"] = _build_bass()
    nc = _CACHE["nc"]

    # host-side fused weights (exact reassociation, done in float64)
    Wn64, We64 = Wn.astype(np.float64), We.astype(np.float64)
    Wo64, Wa64 = Wo.astype(np.float64), Wa.astype(np.float64)
    wno = np.einsum("hdk,hko->hdo", Wn64, Wo64).astype(np.float32)
    weo = np.einsum("hdk,hko->hdo", We64, Wo64).astype(np.float32)
    wna = np.einsum("hdk,hko->hdo", Wn64, Wa64)[..., 0].astype(np.float32)
    wea = np.einsum("hdk,hko->hdo", We64, Wa64)[..., 0].astype(np.float32)

    in_maps = []
    for c in range(N_CORES):
        rows = slice(c * M, (c + 1) * M)
        in_maps.append({
            "inc": np.ascontiguousarray(incidence_matrix[rows]),
            "nf": np.ascontiguousarray(node_features[rows]),
            "ef": edge_features,
            "wno": wno, "weo": weo, "wna": wna, "wea": wea,
        })

    trace = bool(int(os.environ.get("BASS_GNN_TRACE", "0")))
    if trace:
        import importlib.util
        if importlib.util.find_spec("antenv.axon_hooks") is None:
            trace = False
    res = run_bass_kernel_spmd(
        nc, in_maps, core_ids=list(range(N_CORES)), trace=trace)
    _CACHE["last_results"] = res

    out = np.concatenate([res.results[c]["out"] for c in range(N_CORES)], axis=0)
    return out.astype(np.float32)
